# revision 64
# baseline (speedup 1.0000x reference)
"""Trainium2 Bass kernel for nn_Attn_30683246362810 (block-diagonal attention).

Sharding: data-parallel over the 8 equal-length packed sequences
(cu_seqlens = arange*1024) -- core i processes batch i independently,
no collectives.

Per-core pipeline (feature-major activation layout [feature, token]):
  xT (bf16) -> rmsnorm (x^2 on DVE, sum over partitions via ones-matmul,
  rstd broadcast to 128 partitions via a k=1 ones matmul) -> hT (bf16)
  -> QKV matmuls (bf16 weights, pre-transposed + rotary-deinterleave-
  permuted on the host) -> rotary on DVE -> qrot/krot assembly
  (SBUF->SBUF DMAs pack the two rotary halves of each head into one
  contiguous 64-partition band) -> per-head GQA attention with k=64
  score matmuls:
    sc[128tk, 2heads, 512tq] = krot_band^T @ qrot_band  (1 matmul/head)
    exp on ScalarE (no max subtraction -- scores are O(1) here)
    PV with one 66-col stationary [ones|dims|ones] shared by all 4 heads
    of a group: every head's PV psum has the softmax denominator at
    partition 0 and dims at 1..64; gated output is partition-shifted
    into ogT's two 64-row bands by small SBUF->SBUF DMAs.
  -> sigmoid-gate multiply -> out projection (bf16 W_out) to [token, dim].

Emission order interleaves q-group projection/rotation with attention so
PE never idles behind rotations and exps overlap projection matmuls;
outproj of chunk 0 is split across attention of chunk 1. Denominator
reciprocals are broadcast across partitions with k=1 ones-matmuls into
PSUM (GpSimd ucode-library ops fail walrus codegen on this stack).

ScalarE activation-table order per rep: Sqrt -> Sin -> Sigmoid -> Exp
(4 table loads).

Attention/projection operands are bf16 (full PE rate, half SBUF); PSUM
accumulation stays fp32. PSUM: 'sc' ring 2x2 banks (qkv/scores/outproj/
broadcasts) + 'ov' ring 4x1 bank (v-proj, 4 PV accumulators).
"""

import numpy as np

import concourse.bass as bass
import concourse.mybir as mybir
from concourse.tile import TileContext
from concourse.vector_clock import ScopedClock, VectorClock
from concourse.tile_sem_assignment import N_PROCS
from concourse.bass_utils import run_bass_kernel_spmd

F32 = mybir.dt.float32
F32R = mybir.dt.float32r
BF16 = mybir.dt.bfloat16
AF = mybir.ActivationFunctionType
ALU = mybir.AluOpType

N_CORES = 8
T = 1024          # tokens per core (one packed sequence)
D = 1024          # model dim
QH = 16           # query heads
KVH = 4           # kv heads
HD = 64           # head dim
F = HD // 2       # 32 rotary freqs
EPS = 1e-6
SCALE = 1.0 / np.sqrt(HD)
NT = T // 128     # 8 token tiles
ND = D // 128     # 8 dim tiles
NC2 = 2           # token chunks of 512 (fp32 matmul moving-dim max)
CH = 512


class _TC(TileContext):
    """TileContext whose final drain splits its sem waits into 1-wait nops
    (this walrus build rejects >1 sync wait per instruction)."""

    def _drain_and_barrier(self, tick_clock, wait_clock):
        gc = tick_clock.global_clock
        for p in range(N_PROCS):
            t = gc[p]
            if t > 0:
                one = VectorClock([t if q == p else 0 for q in range(N_PROCS)])
                nop = self.nc.sync.add_instruction(
                    mybir.InstNoOp(name=f"I-{self.nc.next_id()}",
                                   engine=mybir.EngineType.SP, bass_nofuse=True))
                wait_clock.add_sem_waits(nop.ins, ScopedClock({None: one}))
        self.nc.sync.drain()
        self.nc.all_engine_barrier()
        assert self.sems is not None
        popped = self.nc._tile_sem_poison_stack.pop()
        assert popped is self._sem_poison
        self.nc.clear_and_free_semaphores(list(self.sems.allocated().values()))
        self.nc.all_engine_barrier()


def _split_multiwaits(nc):
    """Hoist extra sync waits onto preceding same-engine NoOps (1-wait limit)."""
    for f in nc.m.functions:
        for bb in f.blocks:
            insts = list(bb.instructions)
            if not any(i.sync_info is not None and len(i.sync_info.on_wait) > 1
                       for i in insts):
                continue
            new = []
            for i in insts:
                si = i.sync_info
                if si is not None and len(si.on_wait) > 1:
                    waits = list(si.on_wait)
                    for w in waits[:-1]:
                        new.append(mybir.InstNoOp(
                            name=f"I-{nc.next_id()}", engine=i.engine,
                            bass_nofuse=True,
                            sync_info=mybir.SyncInfo(on_wait=[w], on_update=[])))
                    i.sync_info = mybir.SyncInfo(on_wait=[waits[-1]],
                                                 on_update=list(si.on_update))
                new.append(i)
            bb.instructions = new


def build_nc(debug=False, split=True, reps=1):
    nc = bass.Bass("TRN2", dynamic_dma_scratch_size=8192)

    xT_d = nc.dram_tensor("xT", [128, ND, T], BF16, kind="ExternalInput")
    freqsT_d = nc.dram_tensor("freqsT", [F, T], F32, kind="ExternalInput")
    g_d = nc.dram_tensor("g", [D], F32, kind="ExternalInput")
    wqkvT_d = nc.dram_tensor("wqkvT", [20, 128, ND, 128], BF16, kind="ExternalInput")
    woutT_d = nc.dram_tensor("woutT", [128, ND, D], BF16, kind="ExternalInput")
    out_d = nc.dram_tensor("out", [T, D], F32, kind="ExternalOutput")
    dbg = {}
    if debug:
        dbg["hT"] = nc.dram_tensor("dbg_hT", [D, T], BF16, kind="ExternalOutput")
        dbg["og"] = nc.dram_tensor("dbg_og", [D, T], BF16, kind="ExternalOutput")

    with _TC(nc) as tc:
        with (
            tc.tile_pool(name="per", bufs=1) as per,    # persistent (bufs=1/tag)
            tc.tile_pool(name="qkx", bufs=3) as qkx,    # pre-rotary q/k tiles
            tc.tile_pool(name="wstr", bufs=3) as wstr,  # W_qkv streaming
            tc.tile_pool(name="scr", bufs=4) as scr,    # rotary scratch
            tc.tile_pool(name="sq", bufs=1) as sqp,     # x^2 scratch
            tc.tile_pool(name="rqp", bufs=1) as rqp,    # per-group rotated q
            tc.tile_pool(name="pexp", bufs=4) as pexp,  # exp(S^T) tiles
            tc.tile_pool(name="nrm", bufs=2) as nrm,    # recip/broadcast tiles
            tc.tile_pool(name="osc", bufs=1) as oscp,   # odd-head gating scratch
            tc.tile_pool(name="ob", bufs=2) as ob,      # output staging
            tc.tile_pool(name="psc", bufs=2, space="PSUM") as scp,  # 2x2 banks
            tc.tile_pool(name="pov", bufs=4, space="PSUM") as ovp,  # 4x1 bank
        ):
            def _emit(rep):
                debug_r = debug and rep == 0

                def sc_ps():
                    return scp.tile([128, 2, CH], F32, tag="sc", name="sc_ps")

                def ov_ps():
                    return ovp.tile([128, CH], F32, tag="ov", name="ov_ps")

                # ---------------- phase A: load + rmsnorm ----------------
                xT = per.tile([128, ND, T], BF16, tag="xT")
                for j in range(ND):     # per-j split: rmsnorm starts early
                    nc.sync.dma_start(out=xT[:, j, :], in_=xT_d[:, j, :])
                g_sb = per.tile([128, ND], F32, tag="g")
                nc.sync.dma_start(out=g_sb[:], in_=g_d[:].rearrange(
                    "(j p) -> p j", p=128))
                wout_sb = per.tile([128, ND, D], BF16, tag="wout")
                nc.sync.dma_start(out=wout_sb[:], in_=woutT_d[:, :, :])

                ones_f = per.tile([128, 1], F32, tag="ones_f")
                nc.vector.memset(ones_f[:], 1.0)
                ones_col = per.tile([128, 1], F32R, tag="ones")
                nc.vector.tensor_copy(ones_col[:], ones_f[:])
                # ones rows at every partition: lhsT of k=1 broadcast matmuls
                ones128f = per.tile([128, 128], F32, tag="ones128f")
                nc.vector.memset(ones128f[:], 1.0)
                ones128 = per.tile([128, 128], F32R, tag="ones128")
                nc.vector.tensor_copy(ones128[:], ones128f[:])
                eps_sb = per.tile([1, 1], F32, tag="eps")
                nc.vector.memset(eps_sb[:], EPS)

                ssq = per.tile([1, T], F32, tag="ssq")
                ps_s = sc_ps()
                for c in range(NC2):
                    sl = slice(c * CH, (c + 1) * CH)
                    for j in range(ND):
                        xsq = sqp.tile([128, CH], F32R, tag="xsq")
                        # squares on DVE: keeps rep-start PE work ACT-free
                        with nc.allow_low_precision(reason="f32r x^2"):
                            nc.vector.tensor_mul(xsq[:], xT[:, j, sl],
                                                 xT[:, j, sl])
                        nc.tensor.matmul(ps_s[0:1, c, :], ones_col[:], xsq[:],
                                         start=(j == 0), stop=(j == ND - 1))
                nc.vector.tensor_copy(ssq[0:1, :], ps_s[0:1, :, :])

                rstd = per.tile([128, T], F32, tag="rstd")
                nc.scalar.activation(out=rstd[0:1, :], in_=ssq[:], func=AF.Sqrt,
                                     bias=eps_sb[:], scale=1.0 / D)
                rstdr = per.tile([1, T], F32R, tag="rstdr")
                with nc.allow_low_precision(reason="f32r broadcast operand"):
                    nc.vector.reciprocal(rstdr[0:1, :], rstd[0:1, :])
                # broadcast to 128 partitions via k=1 ones matmul
                rstd_b = per.tile([128, T], F32, tag="rstd_b")
                ps_rb = sc_ps()
                for c in range(NC2):
                    nc.tensor.matmul(ps_rb[:, c, :],
                                     ones128[0:1, :],
                                     rstdr[0:1, c * CH:(c + 1) * CH],
                                     start=True, stop=True)
                nc.vector.tensor_copy(rstd_b[:], ps_rb[:])

                hT = per.tile([128, ND, T], BF16, tag="hT")
                for j in range(ND):
                    nc.vector.scalar_tensor_tensor(
                        out=hT[:, j, :], in0=xT[:, j, :], scalar=g_sb[:, j:j + 1],
                        in1=rstd_b[:, :], op0=ALU.mult, op1=ALU.mult)
                if debug_r:
                    nc.sync.dma_start(
                        out=dbg["hT"][:, :].rearrange("(j p) t -> p j t", p=128),
                        in_=hT[:])

                # ------------- rotary cos/sin (fp32, [128, T] = 4x replicated) ----
                freqs128 = scr.tile([128, T], F32, tag="rot")
                nc.gpsimd.dma_start(
                    out=freqs128[:],
                    in_=bass.AP(tensor=freqsT_d[:, :].tensor,
                                offset=freqsT_d[:, :].offset,
                                ap=[[0, 4]] + [list(d) for d in freqsT_d[:, :].ap]))
                S4 = per.tile([128, T], BF16, tag="S4")
                C4 = per.tile([128, T], BF16, tag="C4")
                TWO_PI = float(2 * np.pi)

                def trig(dst, shift):
                    # dst = sin(freqs + shift); ACT Sin domain is [-pi, pi], so
                    # correct by -+2pi where (freqs + shift) leaves it (|arg|<3pi).
                    bias = per.tile([128, 1], F32, tag=f"bias{shift:.2f}",
                                    name="trig_bias")
                    nc.vector.memset(bias[:], float(shift))
                    a = scr.tile([128, T], F32, tag="rot", name="trig_a")
                    nc.vector.tensor_scalar(out=a[:], in0=freqs128[:],
                                            scalar1=float(np.pi - shift), scalar2=None,
                                            op0=ALU.is_ge)
                    b = scr.tile([128, T], F32, tag="rot", name="trig_b")
                    nc.vector.tensor_scalar(out=b[:], in0=freqs128[:],
                                            scalar1=float(-np.pi - shift), scalar2=None,
                                            op0=ALU.is_lt)
                    t1 = scr.tile([128, T], F32, tag="rot", name="trig_t1")
                    nc.vector.scalar_tensor_tensor(
                        out=t1[:], in0=a[:], scalar=-TWO_PI, in1=freqs128[:],
                        op0=ALU.mult, op1=ALU.add)
                    t2 = scr.tile([128, T], F32, tag="rot", name="trig_t2")
                    nc.vector.scalar_tensor_tensor(
                        out=t2[:], in0=b[:], scalar=TWO_PI, in1=t1[:],
                        op0=ALU.mult, op1=ALU.add)
                    nc.scalar.activation(out=dst, in_=t2[:], func=AF.Sin, bias=bias[:])

                trig(S4[:], 0.0)
                trig(C4[:], float(np.pi / 2))

                sg = per.tile([128, ND, T], BF16, tag="sg")
                krot = per.tile([128, KVH, T], BF16, tag="krot")

                def rotate(x1, x2, o1, o2):
                    # o1 = x1*cos - x2*sin ; o2 = x1*sin + x2*cos  (all [128, T])
                    m1 = scr.tile([128, T], F32, tag="rot")
                    m2 = scr.tile([128, T], F32, tag="rot")
                    nc.vector.tensor_mul(m1[:], x1, C4[:])
                    nc.vector.tensor_mul(m2[:], x2, S4[:])
                    nc.vector.tensor_sub(o1, m1[:], m2[:])
                    m3 = scr.tile([128, T], F32, tag="rot")
                    m4 = scr.tile([128, T], F32, tag="rot")
                    nc.vector.tensor_mul(m3[:], x1, S4[:])
                    nc.vector.tensor_mul(m4[:], x2, C4[:])
                    nc.vector.tensor_add(o2, m3[:], m4[:])

                pre = {}

                def emit_qkv_tile(o):
                    """One 128-row output tile of W_qkv @ h, both token chunks.
                    q/k tiles land in bf16 `pre[o]`; gate tiles are sigmoided
                    into sg."""
                    wblk = wstr.tile([128, ND, 128], BF16, tag="wblk",
                                     name="wblk")
                    nc.sync.dma_start(out=wblk[:], in_=wqkvT_d[o])
                    ps_q = sc_ps()
                    # j outer so both chunks share one weight load per j
                    for j in range(ND):
                        for c in range(NC2):
                            sl = slice(c * CH, (c + 1) * CH)
                            nc.tensor.matmul(ps_q[:, c, :], wblk[:, j, :],
                                             hT[:, j, sl],
                                             start=(j == 0), stop=(j == ND - 1))
                    if 8 <= o < 16:
                        nc.scalar.activation(out=sg[:, o - 8, :], in_=ps_q[:],
                                             func=AF.Sigmoid)
                    else:
                        t_pre = qkx.tile([128, T], BF16, tag="qk", name="qk")
                        pre[o] = t_pre
                        nc.vector.tensor_copy(t_pre[:], ps_q[:])

                # ---- k projection + rotation + krot assembly ----
                emit_qkv_tile(16)
                emit_qkv_tile(17)
                rk1 = per.tile([128, T], BF16, tag="rk1")
                rk2 = per.tile([128, T], BF16, tag="rk2")
                rotate(pre[16][:], pre[17][:], rk1[:], rk2[:])
                del pre[16], pre[17]
                # krot[b*64:(b+1)*64, i, :] = [rk1_i ; rk2_i] for both bands b
                for i in range(KVH):
                    for b2 in range(2):
                        nc.sync.dma_start(
                            out=krot[b2 * 64:b2 * 64 + F, i, :],
                            in_=rk1[i * F:(i + 1) * F, :])
                        nc.sync.dma_start(
                            out=krot[b2 * 64 + F:(b2 + 1) * 64, i, :],
                            in_=rk2[i * F:(i + 1) * F, :])

                # ---- v projection (token-major, ones-augmented cols 0 & 65) ----
                v_aug = per.tile([128, NT, KVH, HD + 2], BF16, tag="v_aug")

                def emit_v():
                    nc.vector.memset(v_aug[:, :, :, 0], 1.0)
                    nc.vector.memset(v_aug[:, :, :, HD + 1], 1.0)
                    wv = per.tile([128, ND, 256], BF16, tag="wv")
                    nc.sync.dma_start(out=wv[:, :, 0:128], in_=wqkvT_d[18])
                    nc.sync.dma_start(out=wv[:, :, 128:256], in_=wqkvT_d[19])
                    for tt in range(NT):
                        ps_v = ov_ps()
                        for j in range(ND):
                            nc.tensor.matmul(ps_v[:, 0:256],
                                             hT[:, j, tt * 128:(tt + 1) * 128],
                                             wv[:, j, :],
                                             start=(j == 0), stop=(j == ND - 1))
                        nc.vector.tensor_copy(v_aug[:, tt, :, 1:HD + 1],
                                              ps_v[:, 0:256])

                def emit_attention(i, c):
                    """Attention for kv-group i, token chunk c.

                    Head h = 4i + 2u + P lives in qrot col 2i+u, band P.
                    sc pair P covers heads (4i+P, 4i+2+P). PV uses one 66-col
                    stationary [ones|dims|ones] shared by all 4 heads: every
                    head's PV output has denom at row 0, dims at rows 1..65."""
                    tq = slice(c * CH, (c + 1) * CH)
                    ov = {}
                    for tk in range(NT):
                        tks = slice(tk * 128, (tk + 1) * 128)
                        p_tk = []
                        for P in range(2):
                            bnd = slice(P * 64, (P + 1) * 64)
                            s_ps = sc_ps()
                            for u in range(2):
                                nc.tensor.matmul(
                                    s_ps[:, u, :],
                                    krot[bnd, i, tks],
                                    qrot[bnd, 2 * i + u, tq],
                                    start=True, stop=True,
                                    tile_position=(P * 64, 0))
                            p_sb = pexp.tile([128, 2, CH], BF16,
                                             tag="p_sb", name="p_sb")
                            nc.scalar.activation(out=p_sb[:], in_=s_ps[:],
                                                 func=AF.Exp,
                                                 scale=float(SCALE))
                            p_tk.append(p_sb)
                        stat = v_aug[:, tk, i, 0:HD + 2]
                        for P in range(2):
                            for u in range(2):
                                if tk == 0:
                                    ov[(P, u)] = ov_ps()
                                nc.tensor.matmul(
                                    ov[(P, u)][0:HD + 2, :],
                                    stat,
                                    p_tk[P][:, u, :],
                                    start=(tk == 0), stop=(tk == NT - 1))

                    # normalize + gate: 1/denom broadcast via k=1 ones matmul
                    scrod = oscp.tile([128, 4, CH], BF16, tag="oscr",
                                      name="oscr")
                    rcp = nrm.tile([1, 4, CH], F32R, tag="rcp", name="rcp")
                    with nc.allow_low_precision(
                            reason="f32r broadcast operand"):
                        for P in range(2):
                            for u in range(2):
                                nc.vector.reciprocal(
                                    rcp[0:1, 2 * P + u, :],
                                    ov[(P, u)][0:1, :])
                    rb = nrm.tile([128, 4, CH], F32, tag="rb", name="rb",
                                  bufs=1)
                    for P in range(2):
                        ps_b = sc_ps()
                        for u in range(2):
                            nc.tensor.matmul(ps_b[:, u, :],
                                             ones128[0:1, :],
                                             rcp[0:1, 2 * P + u, :],
                                             start=True, stop=True,
                                             tile_position=(0, 0))
                        # DVE may read only one PSUM input: stage rb in SBUF
                        nc.vector.tensor_copy(rb[0:HD + 1, 2 * P:2 * P + 2, :],
                                              ps_b[0:HD + 1, :, :])
                    for P in range(2):
                        for u in range(2):
                            hidx = 2 * P + u
                            # row 0 is denom*recip == 1.0, sliced off by DMA
                            nc.vector.tensor_mul(scrod[0:HD + 1, hidx, :],
                                                 ov[(P, u)][0:HD + 1, :],
                                                 rb[0:HD + 1, hidx, :])
                    # partition-shift gated dims into ogT's two bands
                    nc.sync.dma_start(out=ogT[0:HD, 2 * i:2 * i + 2, tq],
                                      in_=scrod[1:HD + 1, 0:2, :])
                    nc.sync.dma_start(out=ogT[HD:128, 2 * i:2 * i + 2, tq],
                                      in_=scrod[1:HD + 1, 2:4, :])
                    for b2 in range(2):
                        dstb = ogT[b2 * HD:(b2 + 1) * HD,
                                   2 * i:2 * i + 2, tq]
                        nc.vector.tensor_mul(
                            dstb, dstb,
                            sg[b2 * HD:(b2 + 1) * HD, 2 * i:2 * i + 2, tq])

                def emit_outproj(c, tt2s=(0, 1)):
                    for tt2 in tt2s:              # 2 token tiles per psum buf
                        ps_o = sc_ps()
                        ps_o2 = sc_ps()
                        # j inner-shared: one ogT load feeds both feat chunks
                        for half in range(2):
                            tt = c * 4 + tt2 * 2 + half
                            tts = slice(tt * 128, (tt + 1) * 128)
                            for j in range(ND):
                                nc.tensor.matmul(
                                    ps_o[:, half, :], ogT[:, j, tts],
                                    wout_sb[:, j, 0:CH],
                                    start=(j == 0), stop=(j == ND - 1))
                                nc.tensor.matmul(
                                    ps_o2[:, half, :], ogT[:, j, tts],
                                    wout_sb[:, j, CH:D],
                                    start=(j == 0), stop=(j == ND - 1))
                        for half in range(2):
                            tt = c * 4 + tt2 * 2 + half
                            tts = slice(tt * 128, (tt + 1) * 128)
                            o_sb = ob.tile([128, D], F32, tag="o_sb")
                            nc.vector.tensor_copy(o_sb[:, 0:CH],
                                                  ps_o[:, half, :])
                            nc.vector.tensor_copy(o_sb[:, CH:D],
                                                  ps_o2[:, half, :])
                            nc.sync.dma_start(out=out_d[tts, :], in_=o_sb[:])

                ogT = per.tile([128, ND, T], BF16, tag="ogT")
                qrot = per.tile([128, 2 * KVH, T], BF16, tag="qrot")

                def emit_qgroup(i):
                    emit_qkv_tile(i)
                    emit_qkv_tile(4 + i)
                    rq1 = rqp.tile([128, T], BF16, tag="rq1", name="rq1")
                    rq2 = rqp.tile([128, T], BF16, tag="rq2", name="rq2")
                    rotate(pre[i][:], pre[4 + i][:], rq1[:], rq2[:])
                    del pre[i], pre[4 + i]
                    # head 4i+a -> qrot col 2i + a//2, band a%2
                    for a in range(4):
                        b2, a2 = a % 2, a // 2
                        nc.sync.dma_start(
                            out=qrot[b2 * 64:b2 * 64 + F, 2 * i + a2, :],
                            in_=rq1[a * F:(a + 1) * F, :])
                        nc.sync.dma_start(
                            out=qrot[b2 * 64 + F:(b2 + 1) * 64, 2 * i + a2, :],
                            in_=rq2[a * F:(a + 1) * F, :])

                # ---- v + gates, then q-groups interleaved with attention:
                # PE stays fed during rotations, exps overlap projections ----
                emit_v()
                for o in range(8, 16):
                    emit_qkv_tile(o)
                emit_qgroup(0)
                emit_qgroup(1)
                emit_attention(0, 0)
                emit_qgroup(2)
                emit_attention(1, 0)
                emit_qgroup(3)
                emit_attention(2, 0)
                emit_attention(3, 0)
                emit_attention(0, 1)
                emit_outproj(0, (0,))
                emit_attention(1, 1)
                emit_outproj(0, (1,))
                for i in range(2, KVH):
                    emit_attention(i, 1)
                if debug_r:
                    nc.sync.dma_start(
                        out=dbg["og"][:, :].rearrange("(j p) t -> p j t", p=128),
                        in_=ogT[:])
                emit_outproj(1)

            for _rep in range(reps):
                _emit(_rep)

    if split:
        _split_multiwaits(nc)
    return nc


def _host_prep(x, freqs, g, W_qkv, W_out):
    # W_qkv^T column layout (o): [q_x1 512 | q_x2 512 | gate 1024 |
    #                             k_x1 128 | k_x2 128 | v 256]
    perm = []
    for h in range(QH):
        perm += [h * HD + 2 * f for f in range(F)]
    for h in range(QH):
        perm += [h * HD + 2 * f + 1 for f in range(F)]
    perm += list(range(D, 2 * D))
    for gg in range(KVH):
        perm += [2 * D + gg * HD + 2 * f for f in range(F)]
    for gg in range(KVH):
        perm += [2 * D + gg * HD + 2 * f + 1 for f in range(F)]
    perm += list(range(2 * D + 256, 2 * D + 512))
    import ml_dtypes
    bf16 = ml_dtypes.bfloat16
    wqkvT = np.ascontiguousarray(W_qkv[perm].T, dtype=np.float32)
    # device-tile order: [o_tile, p, j, c] with d = j*128+p, o = o_tile*128+c
    wqkvT = np.ascontiguousarray(
        wqkvT.reshape(8, 128, 20, 128).transpose(2, 1, 0, 3)).astype(bf16)
    woutT = np.ascontiguousarray(
        W_out.T.reshape(8, 128, 1024).transpose(1, 0, 2)).astype(bf16)
    g = np.ascontiguousarray(g, dtype=np.float32)
    in_maps = []
    for ci in range(N_CORES):
        sl = slice(ci * T, (ci + 1) * T)
        in_maps.append({
            "xT": np.ascontiguousarray(
                x[sl].T.reshape(8, 128, 1024).transpose(1, 0, 2)).astype(bf16),
            "freqsT": np.ascontiguousarray(freqs[sl].T, dtype=np.float32),
            "g": g,
            "wqkvT": wqkvT,
            "woutT": woutT,
        })
    return in_maps


_NC_CACHE = {}
_RUNNER_CACHE = {}


def _get_nc(debug=False):
    if debug not in _NC_CACHE:
        _NC_CACHE[debug] = build_nc(debug)
    return _NC_CACHE[debug]


def _make_runner(nc, n_cores=N_CORES):
    """Build a persistent jitted SPMD executor (bass2jax multi-core path)."""
    import jax
    from jax.experimental.shard_map import shard_map
    from jax.sharding import Mesh, PartitionSpec
    from concourse.bass2jax import (_bass_exec_p, install_neuronx_cc_hook,
                                    partition_id_tensor)

    install_neuronx_cc_hook()
    partition_name = (nc.partition_id_tensor.name
                      if nc.partition_id_tensor else None)
    in_names, out_names, out_avals, zero_outs = [], [], [], []
    for alloc in nc.m.functions[0].allocations:
        if not isinstance(alloc, mybir.MemoryLocationSet):
            continue
        name = alloc.memorylocations[0].name
        if alloc.kind == "ExternalInput":
            if name != partition_name:
                in_names.append(name)
        elif alloc.kind == "ExternalOutput":
            shape = tuple(alloc.tensor_shape)
            dtype = mybir.dt.np(alloc.dtype)
            out_names.append(name)
            out_avals.append(jax.core.ShapedArray(shape, dtype))
            zero_outs.append(np.zeros(shape, dtype))
    n_params = len(in_names)
    all_names = list(in_names) + out_names
    if partition_name is not None:
        all_names.append(partition_name)

    def _body(*args):
        operands = list(args)
        if partition_name is not None:
            operands.append(partition_id_tensor())
        outs = _bass_exec_p.bind(
            *operands, out_avals=tuple(out_avals), in_names=tuple(all_names),
            out_names=tuple(out_names), lowering_input_output_aliases=(),
            sim_require_finite=True, sim_require_nnan=True, nc=nc)
        return tuple(outs)

    devices = jax.devices()[:n_cores]
    mesh = Mesh(np.asarray(devices), ("core",))
    n_outs = len(out_names)
    sharded = jax.jit(
        shard_map(_body, mesh=mesh,
                  in_specs=(PartitionSpec("core"),) * (n_params + n_outs),
                  out_specs=(PartitionSpec("core"),) * n_outs,
                  check_rep=False),
        keep_unused=True)

    def run(in_maps):
        per_core = [[np.asarray(m[nm]) for nm in in_names] for m in in_maps]
        concat_in = [np.concatenate([per_core[c][i] for c in range(n_cores)], 0)
                     for i in range(n_params)]
        concat_zero = [np.concatenate([z] * n_cores, 0) for z in zero_outs]
        outs = jax.block_until_ready(sharded(*(concat_in + concat_zero)))
        res = []
        for c in range(n_cores):
            m = {}
            for i, nm in enumerate(out_names):
                per = np.asarray(outs[i])
                sh0 = per.shape[0] // n_cores
                m[nm] = per[c * sh0:(c + 1) * sh0]
            res.append(m)
        return res
    return run


def kernel(x, freqs, g, W_qkv, W_out, cu_seqlens=None, max_seqlen=None,
           _debug=False, _trace=False):
    in_maps = _host_prep(np.asarray(x), np.asarray(freqs), np.asarray(g),
                         np.asarray(W_qkv), np.asarray(W_out))
    nc = _get_nc(_debug)
    if _debug not in _RUNNER_CACHE:
        _RUNNER_CACHE[_debug] = _make_runner(nc)
    results = _RUNNER_CACHE[_debug](in_maps)
    out = np.concatenate([results[ci]["out"] for ci in range(N_CORES)], axis=0)
    if _debug:
        return out, results
    return out


# revision 66
# speedup vs baseline: 1.0659x; 1.0659x over previous
"""Trainium2 Bass kernel for nn_Attn_30683246362810 (block-diagonal attention).

Sharding: data-parallel over the 8 equal-length packed sequences
(cu_seqlens = arange*1024) -- core i processes batch i independently,
no collectives.

Per-core pipeline (feature-major activation layout [feature, token]):
  xT (bf16) -> rmsnorm (x^2 on DVE, sum over partitions via ones-matmul,
  rstd broadcast to 128 partitions via a k=1 ones matmul) -> hT (bf16)
  -> QKV matmuls (bf16 weights, pre-transposed + rotary-deinterleave-
  permuted on the host) -> rotary on DVE -> qrot/krot assembly
  (SBUF->SBUF DMAs pack the two rotary halves of each head into one
  contiguous 64-partition band) -> per-head GQA attention with k=64
  score matmuls:
    sc[128tk, 2heads, 512tq] = krot_band^T @ qrot_band  (1 matmul/head)
    exp on ScalarE (no max subtraction -- scores are O(1) here)
    PV with one 66-col stationary [ones|dims|ones] shared by all 4 heads
    of a group: every head's PV psum has the softmax denominator at
    partition 0 and dims at 1..64; gated output is partition-shifted
    into ogT's two 64-row bands by small SBUF->SBUF DMAs.
  -> sigmoid-gate multiply -> out projection (bf16 W_out) to [token, dim].

Emission order interleaves q-group projection/rotation with attention so
PE never idles behind rotations and exps overlap projection matmuls;
outproj of chunk 0 is split across attention of chunk 1. Denominator
reciprocals are broadcast across partitions with k=1 ones-matmuls into
PSUM (GpSimd ucode-library ops fail walrus codegen on this stack).

ScalarE activation-table order per rep: Sqrt -> Sin -> Sigmoid -> Exp
(4 table loads).

Attention/projection operands are bf16 (full PE rate, half SBUF); PSUM
accumulation stays fp32. PSUM: 'sc' ring 2x2 banks (qkv/scores/outproj/
broadcasts) + 'ov' ring 4x1 bank (v-proj, 4 PV accumulators).
"""

import numpy as np

import concourse.bass as bass
import concourse.mybir as mybir
from concourse.tile import TileContext
from concourse.vector_clock import ScopedClock, VectorClock
from concourse.tile_sem_assignment import N_PROCS
from concourse.bass_utils import run_bass_kernel_spmd

F32 = mybir.dt.float32
F32R = mybir.dt.float32r
BF16 = mybir.dt.bfloat16
AF = mybir.ActivationFunctionType
ALU = mybir.AluOpType

N_CORES = 8
T = 1024          # tokens per core (one packed sequence)
D = 1024          # model dim
QH = 16           # query heads
KVH = 4           # kv heads
HD = 64           # head dim
F = HD // 2       # 32 rotary freqs
EPS = 1e-6
SCALE = 1.0 / np.sqrt(HD)
NT = T // 128     # 8 token tiles
ND = D // 128     # 8 dim tiles
NC2 = 2           # token chunks of 512 (fp32 matmul moving-dim max)
CH = 512


class _TC(TileContext):
    """TileContext whose final drain splits its sem waits into 1-wait nops
    (this walrus build rejects >1 sync wait per instruction)."""

    def _drain_and_barrier(self, tick_clock, wait_clock):
        gc = tick_clock.global_clock
        for p in range(N_PROCS):
            t = gc[p]
            if t > 0:
                one = VectorClock([t if q == p else 0 for q in range(N_PROCS)])
                nop = self.nc.sync.add_instruction(
                    mybir.InstNoOp(name=f"I-{self.nc.next_id()}",
                                   engine=mybir.EngineType.SP, bass_nofuse=True))
                wait_clock.add_sem_waits(nop.ins, ScopedClock({None: one}))
        self.nc.sync.drain()
        self.nc.all_engine_barrier()
        assert self.sems is not None
        popped = self.nc._tile_sem_poison_stack.pop()
        assert popped is self._sem_poison
        self.nc.clear_and_free_semaphores(list(self.sems.allocated().values()))
        self.nc.all_engine_barrier()


def _split_multiwaits(nc):
    """Hoist extra sync waits onto preceding same-engine NoOps (1-wait limit)."""
    for f in nc.m.functions:
        for bb in f.blocks:
            insts = list(bb.instructions)
            if not any(i.sync_info is not None and len(i.sync_info.on_wait) > 1
                       for i in insts):
                continue
            new = []
            for i in insts:
                si = i.sync_info
                if si is not None and len(si.on_wait) > 1:
                    waits = list(si.on_wait)
                    for w in waits[:-1]:
                        new.append(mybir.InstNoOp(
                            name=f"I-{nc.next_id()}", engine=i.engine,
                            bass_nofuse=True,
                            sync_info=mybir.SyncInfo(on_wait=[w], on_update=[])))
                    i.sync_info = mybir.SyncInfo(on_wait=[waits[-1]],
                                                 on_update=list(si.on_update))
                new.append(i)
            bb.instructions = new


def build_nc(debug=False, split=True, reps=1):
    nc = bass.Bass("TRN2", dynamic_dma_scratch_size=8192)

    xT_d = nc.dram_tensor("xT", [128, ND, T], BF16, kind="ExternalInput")
    freqsT_d = nc.dram_tensor("freqsT", [F, T], F32, kind="ExternalInput")
    g_d = nc.dram_tensor("g", [D], F32, kind="ExternalInput")
    wqkvT_d = nc.dram_tensor("wqkvT", [20, 128, ND, 128], BF16, kind="ExternalInput")
    woutT_d = nc.dram_tensor("woutT", [128, ND, D], BF16, kind="ExternalInput")
    out_d = nc.dram_tensor("out", [T, D], F32, kind="ExternalOutput")
    dbg = {}
    if debug:
        dbg["hT"] = nc.dram_tensor("dbg_hT", [D, T], BF16, kind="ExternalOutput")
        dbg["og"] = nc.dram_tensor("dbg_og", [D, T], BF16, kind="ExternalOutput")

    with _TC(nc) as tc:
        with (
            tc.tile_pool(name="per", bufs=1) as per,    # persistent (bufs=1/tag)
            tc.tile_pool(name="qkx", bufs=3) as qkx,    # pre-rotary q/k tiles
            tc.tile_pool(name="wstr", bufs=3) as wstr,  # W_qkv streaming
            tc.tile_pool(name="scr", bufs=4) as scr,    # rotary scratch
            tc.tile_pool(name="sq", bufs=1) as sqp,     # x^2 scratch
            tc.tile_pool(name="rqp", bufs=1) as rqp,    # per-group rotated q
            tc.tile_pool(name="pexp", bufs=4) as pexp,  # exp(S^T) tiles
            tc.tile_pool(name="nrm", bufs=2) as nrm,    # recip/broadcast tiles
            tc.tile_pool(name="osc", bufs=1) as oscp,   # odd-head gating scratch
            tc.tile_pool(name="ob", bufs=2) as ob,      # output staging
            tc.tile_pool(name="psc", bufs=2, space="PSUM") as scp,  # 2x2 banks
            tc.tile_pool(name="pov", bufs=4, space="PSUM") as ovp,  # 4x1 bank
        ):
            def _emit(rep):
                debug_r = debug and rep == 0

                def sc_ps():
                    return scp.tile([128, 2, CH], F32, tag="sc", name="sc_ps")

                def ov_ps():
                    return ovp.tile([128, CH], F32, tag="ov", name="ov_ps")

                # ---------------- phase A: load + rmsnorm ----------------
                xT = per.tile([128, ND, T], BF16, tag="xT")
                for j in range(ND):     # per-j split: rmsnorm starts early
                    nc.sync.dma_start(out=xT[:, j, :], in_=xT_d[:, j, :])
                g_sb = per.tile([128, ND], F32, tag="g")
                nc.sync.dma_start(out=g_sb[:], in_=g_d[:].rearrange(
                    "(j p) -> p j", p=128))
                wout_sb = per.tile([128, ND, D], BF16, tag="wout")
                nc.sync.dma_start(out=wout_sb[:], in_=woutT_d[:, :, :])

                ones_f = per.tile([128, 1], F32, tag="ones_f")
                nc.vector.memset(ones_f[:], 1.0)
                ones_col = per.tile([128, 1], F32R, tag="ones")
                nc.vector.tensor_copy(ones_col[:], ones_f[:])
                # ones rows at every partition: lhsT of k=1 broadcast matmuls
                ones128f = per.tile([128, 128], F32, tag="ones128f")
                nc.vector.memset(ones128f[:], 1.0)
                ones128 = per.tile([128, 128], F32R, tag="ones128")
                nc.vector.tensor_copy(ones128[:], ones128f[:])
                eps_sb = per.tile([1, 1], F32, tag="eps")
                nc.vector.memset(eps_sb[:], EPS)

                ssq = per.tile([1, T], F32, tag="ssq")
                ps_s = sc_ps()
                for c in range(NC2):
                    sl = slice(c * CH, (c + 1) * CH)
                    for j in range(ND):
                        xsq = sqp.tile([128, CH], F32R, tag="xsq")
                        # squares on DVE: keeps rep-start PE work ACT-free
                        with nc.allow_low_precision(reason="f32r x^2"):
                            nc.vector.tensor_mul(xsq[:], xT[:, j, sl],
                                                 xT[:, j, sl])
                        nc.tensor.matmul(ps_s[0:1, c, :], ones_col[:], xsq[:],
                                         start=(j == 0), stop=(j == ND - 1))
                nc.vector.tensor_copy(ssq[0:1, :], ps_s[0:1, :, :])

                rstd = per.tile([128, T], F32, tag="rstd")
                nc.scalar.activation(out=rstd[0:1, :], in_=ssq[:], func=AF.Sqrt,
                                     bias=eps_sb[:], scale=1.0 / D)
                rstdr = per.tile([1, T], F32R, tag="rstdr")
                with nc.allow_low_precision(reason="f32r broadcast operand"):
                    nc.vector.reciprocal(rstdr[0:1, :], rstd[0:1, :])
                # broadcast to 128 partitions via k=1 ones matmul
                rstd_b = per.tile([128, T], F32, tag="rstd_b")
                ps_rb = sc_ps()
                for c in range(NC2):
                    nc.tensor.matmul(ps_rb[:, c, :],
                                     ones128[0:1, :],
                                     rstdr[0:1, c * CH:(c + 1) * CH],
                                     start=True, stop=True)
                nc.vector.tensor_copy(rstd_b[:], ps_rb[:])

                hT = per.tile([128, ND, T], BF16, tag="hT")
                for j in range(ND):
                    nc.vector.scalar_tensor_tensor(
                        out=hT[:, j, :], in0=xT[:, j, :], scalar=g_sb[:, j:j + 1],
                        in1=rstd_b[:, :], op0=ALU.mult, op1=ALU.mult)
                if debug_r:
                    nc.sync.dma_start(
                        out=dbg["hT"][:, :].rearrange("(j p) t -> p j t", p=128),
                        in_=hT[:])

                # ------------- rotary cos/sin (fp32, [128, T] = 4x replicated) ----
                freqs128 = scr.tile([128, T], F32, tag="rot")
                nc.gpsimd.dma_start(
                    out=freqs128[:],
                    in_=bass.AP(tensor=freqsT_d[:, :].tensor,
                                offset=freqsT_d[:, :].offset,
                                ap=[[0, 4]] + [list(d) for d in freqsT_d[:, :].ap]))
                S4 = per.tile([128, T], BF16, tag="S4")
                C4 = per.tile([128, T], BF16, tag="C4")
                TWO_PI = float(2 * np.pi)

                def trig(dst, shift):
                    # dst = sin(freqs + shift); ACT Sin domain is [-pi, pi], so
                    # correct by -+2pi where (freqs + shift) leaves it (|arg|<3pi).
                    bias = per.tile([128, 1], F32, tag=f"bias{shift:.2f}",
                                    name="trig_bias")
                    nc.vector.memset(bias[:], float(shift))
                    a = scr.tile([128, T], F32, tag="rot", name="trig_a")
                    nc.vector.tensor_scalar(out=a[:], in0=freqs128[:],
                                            scalar1=float(np.pi - shift), scalar2=None,
                                            op0=ALU.is_ge)
                    b = scr.tile([128, T], F32, tag="rot", name="trig_b")
                    nc.vector.tensor_scalar(out=b[:], in0=freqs128[:],
                                            scalar1=float(-np.pi - shift), scalar2=None,
                                            op0=ALU.is_lt)
                    t1 = scr.tile([128, T], F32, tag="rot", name="trig_t1")
                    nc.vector.scalar_tensor_tensor(
                        out=t1[:], in0=a[:], scalar=-TWO_PI, in1=freqs128[:],
                        op0=ALU.mult, op1=ALU.add)
                    t2 = scr.tile([128, T], F32, tag="rot", name="trig_t2")
                    nc.vector.scalar_tensor_tensor(
                        out=t2[:], in0=b[:], scalar=TWO_PI, in1=t1[:],
                        op0=ALU.mult, op1=ALU.add)
                    nc.scalar.activation(out=dst, in_=t2[:], func=AF.Sin, bias=bias[:])

                trig(S4[:], 0.0)
                trig(C4[:], float(np.pi / 2))

                sg = per.tile([128, ND, T], BF16, tag="sg")
                krot = per.tile([128, KVH, T], BF16, tag="krot")

                def rotate(x1, x2, o1, o2):
                    # o1 = x1*cos - x2*sin ; o2 = x1*sin + x2*cos  (all [128, T])
                    m1 = scr.tile([128, T], F32, tag="rot")
                    m2 = scr.tile([128, T], F32, tag="rot")
                    nc.vector.tensor_mul(m1[:], x1, C4[:])
                    nc.vector.tensor_mul(m2[:], x2, S4[:])
                    nc.vector.tensor_sub(o1, m1[:], m2[:])
                    m3 = scr.tile([128, T], F32, tag="rot")
                    m4 = scr.tile([128, T], F32, tag="rot")
                    nc.vector.tensor_mul(m3[:], x1, S4[:])
                    nc.vector.tensor_mul(m4[:], x2, C4[:])
                    nc.vector.tensor_add(o2, m3[:], m4[:])

                pre = {}

                def emit_qkv_tile(o):
                    """One 128-row output tile of W_qkv @ h, both token chunks.
                    q/k tiles land in bf16 `pre[o]`; gate tiles are sigmoided
                    into sg."""
                    wblk = wstr.tile([128, ND, 128], BF16, tag="wblk",
                                     name="wblk")
                    nc.sync.dma_start(out=wblk[:], in_=wqkvT_d[o])
                    ps_q = sc_ps()
                    # j outer so both chunks share one weight load per j
                    for j in range(ND):
                        for c in range(NC2):
                            sl = slice(c * CH, (c + 1) * CH)
                            nc.tensor.matmul(ps_q[:, c, :], wblk[:, j, :],
                                             hT[:, j, sl],
                                             start=(j == 0), stop=(j == ND - 1))
                    if 8 <= o < 16:
                        nc.scalar.activation(out=sg[:, o - 8, :], in_=ps_q[:],
                                             func=AF.Sigmoid)
                    else:
                        t_pre = qkx.tile([128, T], BF16, tag="qk", name="qk")
                        pre[o] = t_pre
                        nc.vector.tensor_copy(t_pre[:], ps_q[:])

                # ---- k projection + rotation + krot assembly ----
                emit_qkv_tile(16)
                emit_qkv_tile(17)
                rk1 = per.tile([128, T], BF16, tag="rk1")
                rk2 = per.tile([128, T], BF16, tag="rk2")
                rotate(pre[16][:], pre[17][:], rk1[:], rk2[:])
                del pre[16], pre[17]
                # krot[b*64:(b+1)*64, i, :] = [rk1_i ; rk2_i] for both bands b
                for i in range(KVH):
                    for b2 in range(2):
                        nc.sync.dma_start(
                            out=krot[b2 * 64:b2 * 64 + F, i, :],
                            in_=rk1[i * F:(i + 1) * F, :])
                        nc.sync.dma_start(
                            out=krot[b2 * 64 + F:(b2 + 1) * 64, i, :],
                            in_=rk2[i * F:(i + 1) * F, :])

                # ---- v projection (token-major, ones-augmented cols 0 & 65) ----
                v_aug = per.tile([128, NT, KVH, HD + 2], BF16, tag="v_aug")

                def emit_v():
                    nc.vector.memset(v_aug[:, :, :, 0], 1.0)
                    nc.vector.memset(v_aug[:, :, :, HD + 1], 1.0)
                    wv = per.tile([128, ND, 256], BF16, tag="wv")
                    nc.sync.dma_start(out=wv[:, :, 0:128], in_=wqkvT_d[18])
                    nc.sync.dma_start(out=wv[:, :, 128:256], in_=wqkvT_d[19])
                    for tt in range(NT):
                        ps_v = ov_ps()
                        for j in range(ND):
                            nc.tensor.matmul(ps_v[:, 0:256],
                                             hT[:, j, tt * 128:(tt + 1) * 128],
                                             wv[:, j, :],
                                             start=(j == 0), stop=(j == ND - 1))
                        nc.vector.tensor_copy(v_aug[:, tt, :, 1:HD + 1],
                                              ps_v[:, 0:256])

                def emit_attention(i, c):
                    """Attention for kv-group i, token chunk c.

                    Head h = 4i + 2u + P lives in qrot col 2i+u, band P.
                    sc pair P covers heads (4i+P, 4i+2+P). PV uses one 66-col
                    stationary [ones|dims|ones] shared by all 4 heads: every
                    head's PV output has denom at row 0, dims at rows 1..65."""
                    tq = slice(c * CH, (c + 1) * CH)
                    ov = {}
                    for tk in range(NT):
                        tks = slice(tk * 128, (tk + 1) * 128)
                        p_tk = []
                        for P in range(2):
                            bnd = slice(P * 64, (P + 1) * 64)
                            s_ps = sc_ps()
                            for u in range(2):
                                nc.tensor.matmul(
                                    s_ps[:, u, :],
                                    krot[bnd, i, tks],
                                    qrot[bnd, 2 * i + u, tq],
                                    start=True, stop=True,
                                    tile_position=(P * 64, 0))
                            p_sb = pexp.tile([128, 2, CH], BF16,
                                             tag="p_sb", name="p_sb")
                            nc.scalar.activation(out=p_sb[:], in_=s_ps[:],
                                                 func=AF.Exp,
                                                 scale=float(SCALE))
                            p_tk.append(p_sb)
                        stat = v_aug[:, tk, i, 0:HD + 2]
                        for P in range(2):
                            for u in range(2):
                                if tk == 0:
                                    ov[(P, u)] = ov_ps()
                                nc.tensor.matmul(
                                    ov[(P, u)][0:HD + 2, :],
                                    stat,
                                    p_tk[P][:, u, :],
                                    start=(tk == 0), stop=(tk == NT - 1))

                    # normalize + gate: 1/denom broadcast via k=1 ones matmul
                    scrod = oscp.tile([128, 4, CH], BF16, tag="oscr",
                                      name="oscr")
                    rcp = nrm.tile([1, 4, CH], F32R, tag="rcp", name="rcp")
                    with nc.allow_low_precision(
                            reason="f32r broadcast operand"):
                        for P in range(2):
                            for u in range(2):
                                nc.vector.reciprocal(
                                    rcp[0:1, 2 * P + u, :],
                                    ov[(P, u)][0:1, :])
                    rb = nrm.tile([128, 4, CH], F32, tag="rb", name="rb",
                                  bufs=1)
                    for P in range(2):
                        ps_b = sc_ps()
                        for u in range(2):
                            nc.tensor.matmul(ps_b[:, u, :],
                                             ones128[0:1, :],
                                             rcp[0:1, 2 * P + u, :],
                                             start=True, stop=True,
                                             tile_position=(0, 0))
                        # DVE may read only one PSUM input: stage rb in SBUF
                        nc.vector.tensor_copy(rb[0:HD + 1, 2 * P:2 * P + 2, :],
                                              ps_b[0:HD + 1, :, :])
                    for P in range(2):
                        for u in range(2):
                            hidx = 2 * P + u
                            # row 0 is denom*recip == 1.0, sliced off by DMA
                            nc.vector.tensor_mul(scrod[0:HD + 1, hidx, :],
                                                 ov[(P, u)][0:HD + 1, :],
                                                 rb[0:HD + 1, hidx, :])
                    # partition-shift gated dims into ogT's two bands
                    nc.sync.dma_start(out=ogT[0:HD, 2 * i:2 * i + 2, tq],
                                      in_=scrod[1:HD + 1, 0:2, :])
                    nc.sync.dma_start(out=ogT[HD:128, 2 * i:2 * i + 2, tq],
                                      in_=scrod[1:HD + 1, 2:4, :])
                    for b2 in range(2):
                        dstb = ogT[b2 * HD:(b2 + 1) * HD,
                                   2 * i:2 * i + 2, tq]
                        nc.vector.tensor_mul(
                            dstb, dstb,
                            sg[b2 * HD:(b2 + 1) * HD, 2 * i:2 * i + 2, tq])

                def emit_outproj(c, tt2s=(0, 1)):
                    for tt2 in tt2s:              # 2 token tiles per psum buf
                        ps_o = sc_ps()
                        ps_o2 = sc_ps()
                        # j inner-shared: one ogT load feeds both feat chunks
                        for half in range(2):
                            tt = c * 4 + tt2 * 2 + half
                            tts = slice(tt * 128, (tt + 1) * 128)
                            for j in range(ND):
                                nc.tensor.matmul(
                                    ps_o[:, half, :], ogT[:, j, tts],
                                    wout_sb[:, j, 0:CH],
                                    start=(j == 0), stop=(j == ND - 1))
                                nc.tensor.matmul(
                                    ps_o2[:, half, :], ogT[:, j, tts],
                                    wout_sb[:, j, CH:D],
                                    start=(j == 0), stop=(j == ND - 1))
                        for half in range(2):
                            tt = c * 4 + tt2 * 2 + half
                            tts = slice(tt * 128, (tt + 1) * 128)
                            o_sb = ob.tile([128, D], F32, tag="o_sb")
                            nc.vector.tensor_copy(o_sb[:, 0:CH],
                                                  ps_o[:, half, :])
                            nc.vector.tensor_copy(o_sb[:, CH:D],
                                                  ps_o2[:, half, :])
                            nc.sync.dma_start(out=out_d[tts, :], in_=o_sb[:])

                ogT = per.tile([128, ND, T], BF16, tag="ogT")
                qrot = per.tile([128, 2 * KVH, T], BF16, tag="qrot")

                def emit_qgroup(i):
                    emit_qkv_tile(i)
                    emit_qkv_tile(4 + i)
                    rq1 = rqp.tile([128, T], BF16, tag="rq1", name="rq1")
                    rq2 = rqp.tile([128, T], BF16, tag="rq2", name="rq2")
                    rotate(pre[i][:], pre[4 + i][:], rq1[:], rq2[:])
                    del pre[i], pre[4 + i]
                    # head 4i+a -> qrot col 2i + a//2, band a%2
                    for a in range(4):
                        b2, a2 = a % 2, a // 2
                        nc.sync.dma_start(
                            out=qrot[b2 * 64:b2 * 64 + F, 2 * i + a2, :],
                            in_=rq1[a * F:(a + 1) * F, :])
                        nc.sync.dma_start(
                            out=qrot[b2 * 64 + F:(b2 + 1) * 64, 2 * i + a2, :],
                            in_=rq2[a * F:(a + 1) * F, :])

                # ---- v + gates, then q-groups interleaved with attention:
                # PE stays fed during rotations, exps overlap projections ----
                emit_v()
                for o in range(8, 16):
                    emit_qkv_tile(o)
                emit_qgroup(0)
                emit_qgroup(1)
                emit_attention(0, 0)
                emit_qgroup(2)
                emit_attention(1, 0)
                emit_qgroup(3)
                emit_attention(2, 0)
                emit_attention(3, 0)
                emit_attention(0, 1)
                emit_outproj(0, (0,))
                emit_attention(1, 1)
                emit_outproj(0, (1,))
                for i in range(2, KVH):
                    emit_attention(i, 1)
                if debug_r:
                    nc.sync.dma_start(
                        out=dbg["og"][:, :].rearrange("(j p) t -> p j t", p=128),
                        in_=ogT[:])
                emit_outproj(1)

            for _rep in range(reps):
                _emit(_rep)

    if split:
        _split_multiwaits(nc)
    return nc


def _host_prep(x, freqs, g, W_qkv, W_out):
    # W_qkv^T column layout (o): [q_x1 512 | q_x2 512 | gate 1024 |
    #                             k_x1 128 | k_x2 128 | v 256]
    perm = []
    for h in range(QH):
        perm += [h * HD + 2 * f for f in range(F)]
    for h in range(QH):
        perm += [h * HD + 2 * f + 1 for f in range(F)]
    perm += list(range(D, 2 * D))
    for gg in range(KVH):
        perm += [2 * D + gg * HD + 2 * f for f in range(F)]
    for gg in range(KVH):
        perm += [2 * D + gg * HD + 2 * f + 1 for f in range(F)]
    perm += list(range(2 * D + 256, 2 * D + 512))
    import ml_dtypes
    bf16 = ml_dtypes.bfloat16
    wqkvT = np.ascontiguousarray(W_qkv[perm].T, dtype=np.float32)
    # device-tile order: [o_tile, p, j, c] with d = j*128+p, o = o_tile*128+c
    wqkvT = np.ascontiguousarray(
        wqkvT.reshape(8, 128, 20, 128).transpose(2, 1, 0, 3)).astype(bf16)
    woutT = np.ascontiguousarray(
        W_out.T.reshape(8, 128, 1024).transpose(1, 0, 2)).astype(bf16)
    g = np.ascontiguousarray(g, dtype=np.float32)
    in_maps = []
    for ci in range(N_CORES):
        sl = slice(ci * T, (ci + 1) * T)
        in_maps.append({
            "xT": np.ascontiguousarray(
                x[sl].T.reshape(8, 128, 1024).transpose(1, 0, 2)).astype(bf16),
            "freqsT": np.ascontiguousarray(freqs[sl].T, dtype=np.float32),
            "g": g,
            "wqkvT": wqkvT,
            "woutT": woutT,
        })
    return in_maps


_NC_CACHE = {}
_RUNNER_CACHE = {}


def _get_nc(debug=False):
    if debug not in _NC_CACHE:
        _NC_CACHE[debug] = build_nc(debug)
    return _NC_CACHE[debug]


def _make_runner(nc, n_cores=N_CORES):
    """Build a persistent jitted SPMD executor (bass2jax multi-core path)."""
    import jax
    from jax.experimental.shard_map import shard_map
    from jax.sharding import Mesh, PartitionSpec
    from concourse.bass2jax import (_bass_exec_p, install_neuronx_cc_hook,
                                    partition_id_tensor)

    install_neuronx_cc_hook()
    partition_name = (nc.partition_id_tensor.name
                      if nc.partition_id_tensor else None)
    in_names, out_names, out_avals, zero_outs = [], [], [], []
    for alloc in nc.m.functions[0].allocations:
        if not isinstance(alloc, mybir.MemoryLocationSet):
            continue
        name = alloc.memorylocations[0].name
        if alloc.kind == "ExternalInput":
            if name != partition_name:
                in_names.append(name)
        elif alloc.kind == "ExternalOutput":
            shape = tuple(alloc.tensor_shape)
            dtype = mybir.dt.np(alloc.dtype)
            out_names.append(name)
            out_avals.append(jax.core.ShapedArray(shape, dtype))
            zero_outs.append(np.zeros(shape, dtype))
    n_params = len(in_names)
    all_names = list(in_names) + out_names
    if partition_name is not None:
        all_names.append(partition_name)

    def _body(*args):
        operands = list(args)
        if partition_name is not None:
            operands.append(partition_id_tensor())
        outs = _bass_exec_p.bind(
            *operands, out_avals=tuple(out_avals), in_names=tuple(all_names),
            out_names=tuple(out_names), lowering_input_output_aliases=(),
            sim_require_finite=True, sim_require_nnan=True, nc=nc)
        return tuple(outs)

    devices = jax.devices()[:n_cores]
    mesh = Mesh(np.asarray(devices), ("core",))
    n_outs = len(out_names)
    sharded = jax.jit(
        shard_map(_body, mesh=mesh,
                  in_specs=(PartitionSpec("core"),) * (n_params + n_outs),
                  out_specs=(PartitionSpec("core"),) * n_outs,
                  check_rep=False),
        keep_unused=True)

    def run(in_maps):
        per_core = [[np.asarray(m[nm]) for nm in in_names] for m in in_maps]
        concat_in = [np.concatenate([per_core[c][i] for c in range(n_cores)], 0)
                     for i in range(n_params)]
        concat_zero = [np.concatenate([z] * n_cores, 0) for z in zero_outs]
        outs = jax.block_until_ready(sharded(*(concat_in + concat_zero)))
        res = []
        for c in range(n_cores):
            m = {}
            for i, nm in enumerate(out_names):
                per = np.asarray(outs[i])
                sh0 = per.shape[0] // n_cores
                m[nm] = per[c * sh0:(c + 1) * sh0]
            res.append(m)
        return res
    return run


def kernel(x, freqs, g, W_qkv, W_out, cu_seqlens=None, max_seqlen=None,
           _debug=False, _trace=False):
    in_maps = _host_prep(np.asarray(x), np.asarray(freqs), np.asarray(g),
                         np.asarray(W_qkv), np.asarray(W_out))
    nc = _get_nc(_debug)
    if _debug not in _RUNNER_CACHE:
        _RUNNER_CACHE[_debug] = _make_runner(nc)
    results = _RUNNER_CACHE[_debug](in_maps)
    out = np.concatenate([results[ci]["out"] for ci in range(N_CORES)], axis=0)
    if _debug:
        return out, results
    return out
